# revision 1
# baseline (speedup 1.0000x reference)
"""LlamaAttention (B=1, S=2048, D=2048, H=16, KV=4) on 8 TRN2 NeuronCores.

Tensor-parallel over heads: core c owns q-heads [2c, 2c+1] and kv-head c//2.
Each core computes partial = attn_out_c @ Wo[:, c-slice].T over the full
sequence; the all-reduce after o_proj happens on the host (sum of partials).

Layout strategy: everything on-chip lives feature-on-partitions ("transposed"):
  hsT [d, s], qT/kT/vT [hd, s], attn_outT [hd, s].  The host pre-transposes
hidden_states and weights into partition-major [128, N] arrays so every DMA is
contiguous.  RoPE tables (cos / sign-adjusted sin), the causal diagonal mask
block, and the bf16 identity (for PE transposes) are precomputed on host.

Matmuls run as float32r (TF32-grade, 1 cycle/row at N>=256); P/V in attention
run bf16.  Softmax skips the running-max (scores are O(6) sigma, exp cannot
overflow fp32) and uses the scalar engine's accum_out for the row sums.
"""
import math
import numpy as np

S = 2048
D = 2048
HD = 128
H = 16
KV = 4
NCORES = 8
NT = S // 128          # 16 sequence tiles
DTC = D // 128         # 16 feature chunks
QH = H // NCORES       # 2 q-heads per core
ROPE_BASE = 10000.0
SCALE = 1.0 / math.sqrt(HD)
NEG = -1.0e9

_CACHE = {}


def _rope(nc, pool, dst, src_ps, cos_sb, sin_sb, cols, F32, ALU):
    """dst[:, cols] = src*cos + rotate_half(src)*sin  (src: psum [128, w])."""
    w = cols.stop - cols.start
    tmp = pool.tile([128, w], F32, tag="ropetmp")
    nc.scalar.copy(out=tmp[0:64, :], in_=src_ps[64:128, :])
    nc.scalar.copy(out=tmp[64:128, :], in_=src_ps[0:64, :])
    nc.vector.tensor_tensor(out=dst[:, cols], in0=src_ps, in1=cos_sb[:, cols], op=ALU.mult)
    nc.vector.tensor_tensor(out=tmp, in0=tmp, in1=sin_sb[:, cols], op=ALU.mult)
    nc.vector.tensor_tensor(out=dst[:, cols], in0=dst[:, cols], in1=tmp, op=ALU.add)


def build_nc():
    import concourse.bacc as bacc
    import concourse.tile as tile
    from concourse import mybir

    F32 = mybir.dt.float32
    F32R = mybir.dt.float32r
    BF16 = mybir.dt.bfloat16
    AF = mybir.ActivationFunctionType
    ALU = mybir.AluOpType

    nc = bacc.Bacc("TRN2", target_bir_lowering=False, debug=False)
    hs_d = nc.dram_tensor("hs", [128, DTC * S], F32R, kind="ExternalInput").ap()
    wq_d = nc.dram_tensor("wq", [128, DTC * QH * 128], F32R, kind="ExternalInput").ap()
    wk_d = nc.dram_tensor("wk", [128, DTC * 128], F32R, kind="ExternalInput").ap()
    wv_d = nc.dram_tensor("wv", [128, DTC * 128], F32R, kind="ExternalInput").ap()
    wo_d = nc.dram_tensor("wo", [128, QH * D], F32R, kind="ExternalInput").ap()
    cos_d = nc.dram_tensor("cos", [128, S], F32, kind="ExternalInput").ap()
    sin_d = nc.dram_tensor("sin", [128, S], F32, kind="ExternalInput").ap()
    tri_d = nc.dram_tensor("tri", [128, 128], F32, kind="ExternalInput").ap()
    id_d = nc.dram_tensor("ident", [128, 128], BF16, kind="ExternalInput").ap()
    out_d = nc.dram_tensor("out", [128, NT * D], F32, kind="ExternalOutput").ap()

    hs3 = hs_d.rearrange("p (t s) -> p t s", t=DTC)
    out3 = out_d.rearrange("p (t d) -> p t d", t=NT)

    HS_HALF = S // 2
    NG = NT // 4

    with tile.TileContext(nc) as tc:
        with tc.tile_pool(name="consts", bufs=1) as consts, \
             tc.tile_pool(name="persist", bufs=1) as persist, \
             tc.tile_pool(name="stats", bufs=1) as stats:
            cos_sb = consts.tile([128, S], F32)
            sin_sb = consts.tile([128, S], F32)
            tri_sb = consts.tile([128, 128], F32)
            id_sb = consts.tile([128, 128], BF16)
            wq_sb = consts.tile([128, DTC, QH * 128], F32R)
            wk_sb = consts.tile([128, DTC, 128], F32R)
            wv_sb = consts.tile([128, DTC, 128], F32R)
            wo_sb = consts.tile([128, QH, D], F32R)
            nc.sync.dma_start(out=cos_sb, in_=cos_d)
            nc.sync.dma_start(out=sin_sb, in_=sin_d)
            nc.sync.dma_start(out=tri_sb, in_=tri_d)
            nc.sync.dma_start(out=id_sb, in_=id_d)
            nc.sync.dma_start(out=wq_sb, in_=wq_d.rearrange("p (t m) -> p t m", t=DTC))
            nc.sync.dma_start(out=wk_sb, in_=wk_d.rearrange("p (t m) -> p t m", t=DTC))
            nc.sync.dma_start(out=wv_sb, in_=wv_d.rearrange("p (t m) -> p t m", t=DTC))
            nc.sync.dma_start(out=wo_sb, in_=wo_d.rearrange("p (h m) -> p h m", h=QH))

            qrot = [persist.tile([128, S], F32R, tag=f"qrot{h}", name=f"qrot{h}") for h in range(QH)]
            krot = persist.tile([128, S], F32R, tag="krot")
            vbf = persist.tile([128, S], BF16, tag="vbf")
            vnat = persist.tile([128, NT * 128], BF16, tag="vnat")
            aout = [persist.tile([128, S], F32R, tag=f"aout{h}", name=f"aout{h}") for h in range(QH)]
            l_sb = stats.tile([128, QH * NT], F32, tag="l")
            linv_sb = stats.tile([128, QH * NT], F32, tag="linv")

            # ---------------- QKV projections (+RoPE), s-half at a time -----
            with tc.tile_pool(name="hsp", bufs=2) as hsp, \
                 tc.tile_pool(name="ropet", bufs=2) as ropet, \
                 tc.tile_pool(name="qkvps", bufs=1, space="PSUM") as qkvps:
                for sh in range(2):
                    cols = slice(sh * HS_HALF, (sh + 1) * HS_HALF)
                    pq = [qkvps.tile([128, HS_HALF], F32, tag=f"pq{m}", name=f"pq{m}") for m in range(QH)]
                    pk = qkvps.tile([128, HS_HALF], F32, tag="pk")
                    pv = qkvps.tile([128, HS_HALF], F32, tag="pv")
                    for j in range(DTC // 2):
                        hst = hsp.tile([128, 2, HS_HALF], F32R, tag="hst")
                        nc.sync.dma_start(
                            out=hst,
                            in_=hs3[:, 2 * j:2 * j + 2, sh * HS_HALF:(sh + 1) * HS_HALF])
                        for t2 in range(2):
                            dt = 2 * j + t2
                            st = dt == 0
                            sp = dt == DTC - 1
                            for n in range(HS_HALF // 512):
                                ns = slice(n * 512, (n + 1) * 512)
                                rhs = hst[:, t2, ns]
                                for m in range(QH):
                                    nc.tensor.matmul(pq[m][:, ns],
                                                     wq_sb[:, dt, m * 128:(m + 1) * 128],
                                                     rhs, start=st, stop=sp)
                                nc.tensor.matmul(pk[:, ns], wk_sb[:, dt, :], rhs,
                                                 start=st, stop=sp)
                                nc.tensor.matmul(pv[:, ns], wv_sb[:, dt, :], rhs,
                                                 start=st, stop=sp)
                    for m in range(QH):
                        _rope(nc, ropet, qrot[m], pq[m], cos_sb, sin_sb, cols, F32, ALU)
                    _rope(nc, ropet, krot, pk, cos_sb, sin_sb, cols, F32, ALU)
                    nc.vector.tensor_copy(out=vbf[:, cols], in_=pv)

            # ---------------- attention --------------------------------------
            with tc.tile_pool(name="sps", bufs=1, space="PSUM") as sps, \
                 tc.tile_pool(name="ptps", bufs=2, space="PSUM") as ptps, \
                 tc.tile_pool(name="pvps", bufs=1, space="PSUM") as pvps, \
                 tc.tile_pool(name="pp", bufs=5) as pp, \
                 tc.tile_pool(name="pts", bufs=3) as pts:
                # v: [hd, s] -> natural [s, hd] blocks via PE transpose
                for t4 in range(NT // 4):
                    vt = ptps.tile([128, 512], BF16, tag="pt")
                    for ii in range(4):
                        t = t4 * 4 + ii
                        nc.tensor.transpose(vt[:, ii * 128:(ii + 1) * 128],
                                            vbf[:, t * 128:(t + 1) * 128], id_sb)
                    nc.vector.tensor_copy(out=vnat[:, t4 * 512:(t4 + 1) * 512], in_=vt)

                for g in range(NG):
                    for h in range(QH):
                        ptiles = []
                        for ii in range(4):
                            i = 4 * g + ii
                            width = (i + 1) * 128
                            s_ps = sps.tile([128, S], F32, tag="s")
                            for c0 in range(0, width, 512):
                                ce = min(c0 + 512, width)
                                nc.tensor.matmul(s_ps[:, c0:ce],
                                                 qrot[h][:, i * 128:(i + 1) * 128],
                                                 krot[:, c0:ce], start=True, stop=True)
                            nc.vector.tensor_tensor(out=s_ps[:, i * 128:width],
                                                    in0=s_ps[:, i * 128:width],
                                                    in1=tri_sb, op=ALU.add)
                            p_i = pp.tile([128, S], BF16, tag="p")
                            col = h * NT + i
                            nc.scalar.activation(out=p_i[:, 0:width], in_=s_ps[:, 0:width],
                                                 func=AF.Exp, scale=SCALE,
                                                 accum_out=l_sb[:, col:col + 1])
                            nc.vector.reciprocal(out=linv_sb[:, col:col + 1],
                                                 in_=l_sb[:, col:col + 1])
                            nc.vector.tensor_scalar_mul(p_i[:, 0:width], p_i[:, 0:width],
                                                        linv_sb[:, col:col + 1])
                            ptiles.append((i, width, p_i))
                        pv_ps = pvps.tile([128, 512], F32, tag="pvacc")
                        jmax = 4 * g + 3
                        for j in range(jmax + 1):
                            ii_lo = max(0, j - 4 * g)
                            pt_ps = ptps.tile([128, 512], BF16, tag="pt")
                            for ii in range(ii_lo, 4):
                                i, width, p_i = ptiles[ii]
                                nc.tensor.transpose(pt_ps[:, ii * 128:(ii + 1) * 128],
                                                    p_i[:, j * 128:(j + 1) * 128], id_sb)
                            pt_sb = pts.tile([128, 512], BF16, tag="ptsb")
                            nc.vector.tensor_copy(out=pt_sb[:, ii_lo * 128:512],
                                                  in_=pt_ps[:, ii_lo * 128:512])
                            nc.tensor.matmul(pv_ps[:, ii_lo * 128:512],
                                             vnat[:, j * 128:(j + 1) * 128],
                                             pt_sb[:, ii_lo * 128:512],
                                             start=(j == 0), stop=(j == jmax))
                        nc.vector.tensor_copy(out=aout[h][:, g * 512:(g + 1) * 512],
                                              in_=pv_ps)

            # ---------------- o_proj -----------------------------------------
            with tc.tile_pool(name="ops", bufs=4, space="PSUM") as ops, \
                 tc.tile_pool(name="osb", bufs=2) as osb:
                for t in range(NT):
                    o_sb = osb.tile([128, D], F32, tag="osb")
                    for n in range(D // 512):
                        po = ops.tile([128, 512], F32, tag="po")
                        for h in range(QH):
                            nc.tensor.matmul(po, aout[h][:, t * 128:(t + 1) * 128],
                                             wo_sb[:, h, n * 512:(n + 1) * 512],
                                             start=(h == 0), stop=(h == QH - 1))
                        nc.scalar.copy(out=o_sb[:, n * 512:(n + 1) * 512], in_=po)
                    nc.sync.dma_start(out=out3[:, t, :], in_=o_sb)

    nc.compile()
    return nc


def _pm(x):
    """[n*128, M] row-major -> partition-major [128, n*M]."""
    n = x.shape[0] // 128
    return np.ascontiguousarray(
        x.reshape(n, 128, x.shape[1]).transpose(1, 0, 2).reshape(128, -1))


def prep_in_maps(hidden_states, position_ids, Wq, Wk, Wv, Wo):
    import ml_dtypes
    hs = np.asarray(hidden_states, dtype=np.float32).reshape(S, D)
    hsT_pm = _pm(np.ascontiguousarray(hs.T))                       # [128, DTC*S]

    pos = np.asarray(position_ids).reshape(S).astype(np.float32)
    inv = (ROPE_BASE ** (-np.arange(0, HD, 2, dtype=np.float32) / HD))  # [64]
    ang = np.concatenate([pos[None, :] * inv[:, None]] * 2, axis=0)     # [128, S]
    cos_t = np.cos(ang).astype(np.float32)
    sin_t = np.sin(ang).astype(np.float32)
    sin_signed = np.concatenate([-sin_t[:64], sin_t[64:]], axis=0)

    q_idx = np.arange(128)[:, None]
    k_idx = np.arange(128)[None, :]
    tri = np.where(k_idx <= q_idx, 0.0, NEG).astype(np.float32)
    ident = np.eye(128, dtype=ml_dtypes.bfloat16)

    Wq = np.asarray(Wq, np.float32)
    Wk = np.asarray(Wk, np.float32)
    Wv = np.asarray(Wv, np.float32)
    Wo = np.asarray(Wo, np.float32)

    in_maps = []
    for c in range(NCORES):
        g = (c * QH) // (H // KV)          # kv head owned by this core
        wq_c = Wq[c * QH * 128:(c + 1) * QH * 128]      # [256, D]
        wk_c = Wk[g * 128:(g + 1) * 128]                # [128, D]
        wv_c = Wv[g * 128:(g + 1) * 128]                # [128, D]
        wo_c = Wo[:, c * QH * 128:(c + 1) * QH * 128]   # [D, 256]
        in_maps.append({
            "hs": hsT_pm,
            "wq": _pm(np.ascontiguousarray(wq_c.T)),
            "wk": _pm(np.ascontiguousarray(wk_c.T)),
            "wv": _pm(np.ascontiguousarray(wv_c.T)),
            "wo": _pm(np.ascontiguousarray(wo_c.T)),
            "cos": cos_t,
            "sin": sin_signed,
            "tri": tri,
            "ident": ident,
        })
    return in_maps


def combine_outputs(results):
    total = np.zeros((S, D), np.float32)
    for r in results:
        o = np.asarray(r["out"], np.float32)
        total += o.reshape(128, NT, D).transpose(1, 0, 2).reshape(S, D)
    return total[None]


def kernel(hidden_states, attention_mask, position_ids, Wq, Wk, Wv, Wo):
    from concourse import bass_utils
    if "nc" not in _CACHE:
        _CACHE["nc"] = build_nc()
    nc = _CACHE["nc"]
    in_maps = prep_in_maps(hidden_states, position_ids, Wq, Wk, Wv, Wo)
    res = bass_utils.run_bass_kernel_spmd(nc, in_maps, core_ids=list(range(NCORES)))
    return combine_outputs(res.results)



# revision 2
# speedup vs baseline: 1.3184x; 1.3184x over previous
"""LlamaAttention (B=1, S=2048, D=2048, H=16, KV=4) on 8 TRN2 NeuronCores.

Tensor-parallel over heads: core c owns q-heads [2c, 2c+1] and kv-head c//2.
Each core computes partial = attn_out_c @ Wo[:, c-slice].T over the full
sequence; the all-reduce after o_proj happens on the host (sum of partials).

Layout strategy: everything on-chip lives feature-on-partitions ("transposed"):
  hsT [d, s], qT/kT/vT [hd, s], attn_outT [hd, s].  The host pre-transposes
hidden_states and weights into partition-major [128, N] arrays so every DMA is
contiguous.  RoPE tables (cos / sign-adjusted sin), the causal diagonal mask
block, and the bf16 identity (for PE transposes) are precomputed on host.

v2: all matmuls run bf16 (stationary+moving) so the compiler's fast-weight-load
kicks in and HBM traffic halves; weights stream per-dt chunk so compute starts
within a few us; attention is chunked into 512-wide PSUM tiles and
software-pipelined across (group, head) units; o_proj is fused into the
attention loop per group and writes bf16 partials (host all-reduces in fp32).
"""
import math
import numpy as np

S = 2048
D = 2048
HD = 128
H = 16
KV = 4
NCORES = 8
NT = S // 128          # 16 sequence tiles
DTC = D // 128         # 16 feature chunks
QH = H // NCORES       # 2 q-heads per core
ROPE_BASE = 10000.0
SCALE = 1.0 / math.sqrt(HD)
NEG = -1.0e9

_CACHE = {}


def _rope(nc, pool, dst, src_ps, cos_sb, sin_sb, cols, F32, ALU):
    """dst[:, cols] = src*cos + rotate_half(src)*sin  (src: psum [128, w])."""
    w = cols.stop - cols.start
    tmp = pool.tile([128, w], F32, tag="ropetmp")
    t1 = pool.tile([128, w], F32, tag="ropet1")
    nc.scalar.copy(out=tmp[0:64, :], in_=src_ps[64:128, :])
    nc.scalar.copy(out=tmp[64:128, :], in_=src_ps[0:64, :])
    nc.vector.tensor_tensor(out=t1, in0=src_ps, in1=cos_sb[:, cols], op=ALU.mult)
    nc.vector.tensor_tensor(out=tmp, in0=tmp, in1=sin_sb[:, cols], op=ALU.mult)
    nc.vector.tensor_tensor(out=dst[:, cols], in0=t1, in1=tmp, op=ALU.add)


def build_nc():
    import concourse.bacc as bacc
    import concourse.tile as tile
    from concourse import mybir

    F32 = mybir.dt.float32
    BF16 = mybir.dt.bfloat16
    AF = mybir.ActivationFunctionType
    ALU = mybir.AluOpType

    nc = bacc.Bacc("TRN2", target_bir_lowering=False, debug=False)
    hs_d = nc.dram_tensor("hs", [128, DTC * S], BF16, kind="ExternalInput").ap()
    wq_d = nc.dram_tensor("wq", [128, DTC * QH * 128], BF16, kind="ExternalInput").ap()
    wk_d = nc.dram_tensor("wk", [128, DTC * 128], BF16, kind="ExternalInput").ap()
    wv_d = nc.dram_tensor("wv", [128, DTC * 128], BF16, kind="ExternalInput").ap()
    wo_d = nc.dram_tensor("wo", [128, QH * D], BF16, kind="ExternalInput").ap()
    cos_d = nc.dram_tensor("cos", [128, S], F32, kind="ExternalInput").ap()
    sin_d = nc.dram_tensor("sin", [128, S], F32, kind="ExternalInput").ap()
    tri_d = nc.dram_tensor("tri", [128, 128], F32, kind="ExternalInput").ap()
    id_d = nc.dram_tensor("ident", [128, 128], BF16, kind="ExternalInput").ap()
    out_d = nc.dram_tensor("out", [128, NT * D], BF16, kind="ExternalOutput").ap()

    hs3 = hs_d.rearrange("p (t s) -> p t s", t=DTC)
    wq3 = wq_d.rearrange("p (t m) -> p t m", t=DTC)
    wk3 = wk_d.rearrange("p (t m) -> p t m", t=DTC)
    wv3 = wv_d.rearrange("p (t m) -> p t m", t=DTC)
    out3 = out_d.rearrange("p (t d) -> p t d", t=NT)

    HS_HALF = S // 2

    with tile.TileContext(nc) as tc:
        with tc.tile_pool(name="consts", bufs=1) as consts, \
             tc.tile_pool(name="persist", bufs=1) as persist, \
             tc.tile_pool(name="stats", bufs=1) as stats:
            tri_sb = consts.tile([128, 128], F32)
            id_sb = consts.tile([128, 128], BF16)
            cos_sb = consts.tile([128, S], F32)
            sin_sb = consts.tile([128, S], F32)
            wq_sb = consts.tile([128, DTC, QH * 128], BF16)
            wk_sb = consts.tile([128, DTC, 128], BF16)
            wv_sb = consts.tile([128, DTC, 128], BF16)
            wo_sb = consts.tile([128, QH, D], BF16)
            # tiny consts first, then per-dt weight chunks so the first
            # matmul only waits for chunk 0, then rope tables / wo.
            nc.sync.dma_start(out=tri_sb, in_=tri_d)
            nc.sync.dma_start(out=id_sb, in_=id_d)
            for dt in range(DTC):
                nc.sync.dma_start(out=wq_sb[:, dt, :], in_=wq3[:, dt, :])
                nc.sync.dma_start(out=wk_sb[:, dt, :], in_=wk3[:, dt, :])
                nc.sync.dma_start(out=wv_sb[:, dt, :], in_=wv3[:, dt, :])
            nc.sync.dma_start(out=cos_sb, in_=cos_d)
            nc.sync.dma_start(out=sin_sb, in_=sin_d)
            nc.sync.dma_start(out=wo_sb, in_=wo_d.rearrange("p (h m) -> p h m", h=QH))

            qrot = [persist.tile([128, S], BF16, tag=f"qrot{h}", name=f"qrot{h}") for h in range(QH)]
            krot = persist.tile([128, S], BF16, tag="krot")
            vbf = persist.tile([128, S], BF16, tag="vbf")
            vnat = persist.tile([128, NT * 128], BF16, tag="vnat")
            aout = [persist.tile([128, S], BF16, tag=f"aout{h}", name=f"aout{h}") for h in range(QH)]
            l_sb = stats.tile([128, QH * NT], F32, tag="l")
            linv_sb = stats.tile([128, QH * NT], F32, tag="linv")
            lpart = stats.tile([128, QH * NT * 4], F32, tag="lpart")

            # ---------------- QKV projections (+RoPE), s-half at a time -----
            with tc.tile_pool(name="hsp", bufs=3) as hsp, \
                 tc.tile_pool(name="ropet", bufs=2) as ropet, \
                 tc.tile_pool(name="qkvps", bufs=1, space="PSUM") as qkvps:
                for sh in range(2):
                    cols = slice(sh * HS_HALF, (sh + 1) * HS_HALF)
                    pq = [qkvps.tile([128, HS_HALF], F32, tag=f"pq{m}", name=f"pq{m}") for m in range(QH)]
                    pk = qkvps.tile([128, HS_HALF], F32, tag="pk")
                    pv = qkvps.tile([128, HS_HALF], F32, tag="pv")
                    for j in range(DTC // 2):
                        hst = hsp.tile([128, 2, HS_HALF], BF16, tag="hst")
                        nc.sync.dma_start(
                            out=hst,
                            in_=hs3[:, 2 * j:2 * j + 2, sh * HS_HALF:(sh + 1) * HS_HALF])
                        for t2 in range(2):
                            dt = 2 * j + t2
                            st = dt == 0
                            sp = dt == DTC - 1
                            wlist = ([(wq_sb[:, dt, m * 128:(m + 1) * 128], pq[m]) for m in range(QH)]
                                     + [(wk_sb[:, dt, :], pk), (wv_sb[:, dt, :], pv)])
                            for w_ap, dst in wlist:
                                for n in range(HS_HALF // 512):
                                    ns = slice(n * 512, (n + 1) * 512)
                                    nc.tensor.matmul(dst[:, ns], w_ap, hst[:, t2, ns],
                                                     start=st, stop=sp)
                    for m in range(QH):
                        _rope(nc, ropet, qrot[m], pq[m], cos_sb, sin_sb, cols, F32, ALU)
                    _rope(nc, ropet, krot, pk, cos_sb, sin_sb, cols, F32, ALU)
                    nc.vector.tensor_copy(out=vbf[:, cols], in_=pv)

            # ---------------- attention + fused o_proj ------------------------
            with tc.tile_pool(name="sps", bufs=4, space="PSUM") as sps, \
                 tc.tile_pool(name="ptps", bufs=2, space="PSUM") as ptps, \
                 tc.tile_pool(name="pvps", bufs=2, space="PSUM") as pvps, \
                 tc.tile_pool(name="pp", bufs=10) as pp, \
                 tc.tile_pool(name="pts", bufs=4) as pts, \
                 tc.tile_pool(name="osb", bufs=2) as osb:
                # v: [hd, s] -> natural [s, hd] blocks via PE transpose
                for t4 in range(NT // 4):
                    vt = ptps.tile([128, 512], BF16, tag="pt")
                    for ii in range(4):
                        t = t4 * 4 + ii
                        nc.tensor.transpose(vt[:, ii * 128:(ii + 1) * 128],
                                            vbf[:, t * 128:(t + 1) * 128], id_sb)
                    nc.vector.tensor_copy(out=vnat[:, t4 * 512:(t4 + 1) * 512], in_=vt)

                units = [(g, h) for g in range(NT // 4) for h in range(QH)]

                def stage_scores(u):
                    """QK chunks + mask + exp + normalize for unit u; returns ptiles."""
                    g, h = units[u]
                    ptiles = []
                    for ii in range(4):
                        i = 4 * g + ii
                        W = (i + 1) * 128
                        p_i = pp.tile([128, S], BF16, tag="p", name=f"p{u}_{ii}")
                        col = h * NT + i
                        nch = (W + 511) // 512
                        for c in range(nch):
                            c0 = 512 * c
                            ce = min(c0 + 512, W)
                            s_ch = sps.tile([128, 512], F32, tag="s")
                            nc.tensor.matmul(s_ch[:, 0:ce - c0],
                                             qrot[h][:, i * 128:(i + 1) * 128],
                                             krot[:, c0:ce], start=True, stop=True)
                            if ce == W:   # diagonal block lives in this chunk
                                nc.vector.tensor_tensor(
                                    out=s_ch[:, W - 128 - c0:W - c0],
                                    in0=s_ch[:, W - 128 - c0:W - c0],
                                    in1=tri_sb, op=ALU.add)
                            nc.scalar.activation(out=p_i[:, c0:ce], in_=s_ch[:, 0:ce - c0],
                                                 func=AF.Exp, scale=SCALE,
                                                 accum_out=lpart[:, col * 4 + c:col * 4 + c + 1])
                        if nch > 1:
                            nc.vector.tensor_reduce(out=l_sb[:, col:col + 1],
                                                    in_=lpart[:, col * 4:col * 4 + nch],
                                                    axis=mybir.AxisListType.X, op=ALU.add)
                            nc.vector.reciprocal(out=linv_sb[:, col:col + 1],
                                                 in_=l_sb[:, col:col + 1])
                        else:
                            nc.vector.reciprocal(out=linv_sb[:, col:col + 1],
                                                 in_=lpart[:, col * 4:col * 4 + 1])
                        nc.vector.tensor_scalar_mul(p_i[:, 0:W], p_i[:, 0:W],
                                                    linv_sb[:, col:col + 1])
                        ptiles.append((i, W, p_i))
                    return ptiles

                def stage_pv(u, ptiles):
                    """transpose-pipelined P@V accumulation + aout; o_proj per group."""
                    g, h = units[u]
                    jmax = 4 * g + 3

                    def do_transpose(j):
                        ii_lo = max(0, j - 4 * g)
                        ptp = ptps.tile([128, 512], BF16, tag="pt")
                        for ii in range(ii_lo, 4):
                            i, W, p_i = ptiles[ii]
                            nc.tensor.transpose(ptp[:, ii * 128:(ii + 1) * 128],
                                                p_i[:, j * 128:(j + 1) * 128], id_sb)
                        return ptp

                    pv_ps = pvps.tile([128, 512], F32, tag="pv")
                    ptp_cur = do_transpose(0)
                    for j in range(jmax + 1):
                        ii_lo = max(0, j - 4 * g)
                        pt_sb = pts.tile([128, 512], BF16, tag="ptsb")
                        nc.vector.tensor_copy(out=pt_sb[:, ii_lo * 128:512],
                                              in_=ptp_cur[:, ii_lo * 128:512])
                        if j < jmax:   # keep PE busy while the copy drains
                            ptp_cur = do_transpose(j + 1)
                        nc.tensor.matmul(pv_ps[:, ii_lo * 128:512],
                                         vnat[:, j * 128:(j + 1) * 128],
                                         pt_sb[:, ii_lo * 128:512],
                                         start=(j == 0), stop=(j == jmax))
                    nc.vector.tensor_copy(out=aout[h][:, g * 512:(g + 1) * 512],
                                          in_=pv_ps)
                    if h == QH - 1:   # both heads of group g done -> o_proj its tiles
                        for t in range(4 * g, 4 * g + 4):
                            o_sb = osb.tile([128, D], BF16, tag="osb")
                            for n in range(D // 512):
                                po = pvps.tile([128, 512], F32, tag="pv", name=f"po{t}_{n}")
                                for hh in range(QH):
                                    nc.tensor.matmul(po, aout[hh][:, t * 128:(t + 1) * 128],
                                                     wo_sb[:, hh, n * 512:(n + 1) * 512],
                                                     start=(hh == 0), stop=(hh == QH - 1))
                                nc.vector.tensor_copy(out=o_sb[:, n * 512:(n + 1) * 512], in_=po)
                            nc.sync.dma_start(out=out3[:, t, :], in_=o_sb)

                # software pipeline across units: scores(u+1) issued before pv(u)
                ptiles_u = stage_scores(0)
                for u in range(len(units)):
                    ptiles_next = stage_scores(u + 1) if u + 1 < len(units) else None
                    stage_pv(u, ptiles_u)
                    ptiles_u = ptiles_next

    nc.compile()
    return nc


def _pm(x):
    """[n*128, M] row-major -> partition-major [128, n*M]."""
    n = x.shape[0] // 128
    return np.ascontiguousarray(
        x.reshape(n, 128, x.shape[1]).transpose(1, 0, 2).reshape(128, -1))


def prep_in_maps(hidden_states, position_ids, Wq, Wk, Wv, Wo):
    import ml_dtypes
    BF = ml_dtypes.bfloat16
    hs = np.asarray(hidden_states, dtype=np.float32).reshape(S, D)
    hsT_pm = _pm(np.ascontiguousarray(hs.T)).astype(BF)             # [128, DTC*S]

    pos = np.asarray(position_ids).reshape(S).astype(np.float32)
    inv = (ROPE_BASE ** (-np.arange(0, HD, 2, dtype=np.float32) / HD))  # [64]
    ang = np.concatenate([pos[None, :] * inv[:, None]] * 2, axis=0)     # [128, S]
    cos_t = np.cos(ang).astype(np.float32)
    sin_t = np.sin(ang).astype(np.float32)
    sin_signed = np.concatenate([-sin_t[:64], sin_t[64:]], axis=0)

    q_idx = np.arange(128)[:, None]
    k_idx = np.arange(128)[None, :]
    tri = np.where(k_idx <= q_idx, 0.0, NEG).astype(np.float32)
    ident = np.eye(128, dtype=BF)

    Wq = np.asarray(Wq, np.float32)
    Wk = np.asarray(Wk, np.float32)
    Wv = np.asarray(Wv, np.float32)
    Wo = np.asarray(Wo, np.float32)

    in_maps = []
    for c in range(NCORES):
        g = (c * QH) // (H // KV)          # kv head owned by this core
        wq_c = Wq[c * QH * 128:(c + 1) * QH * 128]      # [256, D]
        wk_c = Wk[g * 128:(g + 1) * 128]                # [128, D]
        wv_c = Wv[g * 128:(g + 1) * 128]                # [128, D]
        wo_c = Wo[:, c * QH * 128:(c + 1) * QH * 128]   # [D, 256]
        in_maps.append({
            "hs": hsT_pm,
            "wq": _pm(np.ascontiguousarray(wq_c.T)).astype(BF),
            "wk": _pm(np.ascontiguousarray(wk_c.T)).astype(BF),
            "wv": _pm(np.ascontiguousarray(wv_c.T)).astype(BF),
            "wo": _pm(np.ascontiguousarray(wo_c.T)).astype(BF),
            "cos": cos_t,
            "sin": sin_signed,
            "tri": tri,
            "ident": ident,
        })
    return in_maps


def combine_outputs(results):
    total = np.zeros((S, D), np.float32)
    for r in results:
        o = np.asarray(r["out"], np.float32)
        total += o.reshape(128, NT, D).transpose(1, 0, 2).reshape(S, D)
    return total[None]


def kernel(hidden_states, attention_mask, position_ids, Wq, Wk, Wv, Wo):
    from concourse import bass_utils
    if "nc" not in _CACHE:
        _CACHE["nc"] = build_nc()
    nc = _CACHE["nc"]
    in_maps = prep_in_maps(hidden_states, position_ids, Wq, Wk, Wv, Wo)
    res = bass_utils.run_bass_kernel_spmd(nc, in_maps, core_ids=list(range(NCORES)))
    return combine_outputs(res.results)
